# revision 3
# baseline (speedup 1.0000x reference)
"""Trainium2 Bass kernel for nn_BackgroundNoiseLayer.

Math: out[t, n*5+r] = sum_k spikes[t,k] * Wr[k, n*5+r]
  spikes (600,100) binary, from rest_of_brain < 0.25
  Wr (100, 200000) = scatter-add of edge values (host-side index preprocessing)

Distribution: 1D column-parallel over the 8 cores — each core gets a
25000-wide slab of Wr (its 5000 post-neurons x 5 receptors), spikes
replicated; per-core output slab (600, 25000) is concatenated on host.

Device kernel (SPMD, identical on all cores): load Wr slab (10 MB) + spikesT
into SBUF once, then for each of 5 token tiles (128,128,128,128,88) stream 50
fp32 matmuls (K=100, N=500) into PSUM, copy PSUM->SBUF staging (alternating
DVE/ACT), and DMA 2.5 MB staging blocks to the DRAM output. The kernel is
HBM-write-bound (60 MB out per core), matching target_regime=memory.
"""

import numpy as np

import concourse.bass as bass
import concourse.mybir as mybir
import concourse.tile as tile
from concourse.bass_utils import run_bass_kernel_spmd

# ---------------------------------------------------------------------------
# Workaround for walrus codegen limit on this toolchain: an instruction with
# more than one sync wait fails codegen ("Too many sync wait commands").
# Split every multi-wait instruction: extra waits move to single-wait NoOps
# inserted just before it on the same engine queue (same-engine FIFO dispatch
# preserves gating semantics).
# ---------------------------------------------------------------------------
def _split_multi_waits(nc):
    n_split = 0
    for fn in nc.m.functions:
        for bb in fn.blocks:
            new_list = []
            for inst in bb.instructions:
                si = inst.sync_info
                waits = list(si.on_wait) if si is not None and si.on_wait else []
                if len(waits) > 1:
                    for j, w in enumerate(waits[:-1]):
                        nop = mybir.InstNoOp(
                            name=f"{inst.name}_w{j}", ins=[], outs=[]
                        )
                        nop.engine = inst.engine
                        nop.sync_info = mybir.SyncInfo(on_wait=[w], on_update=[])
                        new_list.append(nop)
                        n_split += 1
                    inst.sync_info = mybir.SyncInfo(
                        on_wait=[waits[-1]], on_update=list(si.on_update or [])
                    )
                new_list.append(inst)
            bb.instructions = new_list
    return n_split

# ---------------------------------------------------------------------------
# Problem constants (hardcoded; kernel.py must be self-contained)
# ---------------------------------------------------------------------------
N_NEURONS = 40000
N_BKG = 100          # K (contraction dim)
NNZ = 200000
N_RECEPTOR_ROWS = 10
N_SYN_BASIS = 5
T = 600              # BATCH * SEQ tokens
N_CORES = 8
NR = N_NEURONS * N_SYN_BASIS          # 200000 output columns
NR_CORE = NR // N_CORES               # 25000 per core

T_TILES = [128, 128, 128, 128, 88]    # sum = 600
CHUNK = 500                           # matmul N (<=512 fp32, one PSUM bank)
GROUP = 5000                          # staging width = 10 chunks
N_GROUPS = NR_CORE // GROUP           # 5
F32 = mybir.dt.float32

_NC_CACHE = None


def _build_nc():
    nc = bass.Bass()
    spikes_t = nc.dram_tensor("spikes_t", [N_BKG, T], F32, kind="ExternalInput")
    wr = nc.dram_tensor("wr", [N_BKG, NR_CORE], F32, kind="ExternalInput")
    out = nc.dram_tensor("out", [T, NR_CORE], F32, kind="ExternalOutput")

    with tile.TileContext(nc) as tc:
        with (
            tc.tile_pool(name="wpool", bufs=1) as wpool,
            tc.tile_pool(name="spool", bufs=1) as spool,
            tc.tile_pool(name="stage", bufs=3) as stage,
            tc.tile_pool(name="psum", bufs=8, space="PSUM") as psum,
        ):
            sp_sb = spool.tile([N_BKG, T], F32)
            nc.sync.dma_start(sp_sb[:], spikes_t[:])
            wr_sb = wpool.tile([N_BKG, NR_CORE], F32)
            for g in range(N_GROUPS):
                nc.sync.dma_start(
                    wr_sb[:, g * GROUP : (g + 1) * GROUP],
                    wr[:, g * GROUP : (g + 1) * GROUP],
                )

            copy_i = 0
            for ti, m in enumerate(T_TILES):
                t0 = ti * 128
                for g in range(N_GROUPS):
                    st = stage.tile([m, GROUP], F32)
                    for c in range(GROUP // CHUNK):
                        col = g * GROUP + c * CHUNK
                        ps = psum.tile([m, CHUNK], F32)
                        nc.tensor.matmul(
                            ps[:],
                            sp_sb[:, t0 : t0 + m],
                            wr_sb[:, col : col + CHUNK],
                            start=True,
                            stop=True,
                        )
                        dst = st[:, c * CHUNK : (c + 1) * CHUNK]
                        if copy_i % 2 == 0:
                            nc.vector.tensor_copy(dst, ps[:])
                        else:
                            nc.scalar.copy(dst, ps[:])
                        copy_i += 1
                    nc.scalar.dma_start(
                        out[t0 : t0 + m, g * GROUP : (g + 1) * GROUP], st[:]
                    )
    _split_multi_waits(nc)
    return nc


def get_nc():
    global _NC_CACHE
    if _NC_CACHE is None:
        _NC_CACHE = _build_nc()
    return _NC_CACHE


def _host_preprocess(weights, synaptic_weights, rest_of_brain, post_idx, pre_idx,
                     syn_ids):
    spikes = (rest_of_brain.reshape(T, N_BKG) < 0.25).astype(np.float32)
    spikes_t = np.ascontiguousarray(spikes.T)                      # (100, 600)

    vals = weights[:, None] * synaptic_weights[syn_ids]            # (nnz, 5)
    cell = post_idx.astype(np.int64) * N_BKG + pre_idx.astype(np.int64)
    flat = (cell[:, None] * N_SYN_BASIS + np.arange(N_SYN_BASIS)[None, :]).ravel()
    w_dense = np.bincount(
        flat, weights=vals.astype(np.float64).ravel(),
        minlength=N_NEURONS * N_BKG * N_SYN_BASIS,
    ).astype(np.float32).reshape(N_NEURONS, N_BKG, N_SYN_BASIS)
    # Wr[k, n*5+r] = W[n, k, r]
    wr_full = np.ascontiguousarray(w_dense.transpose(1, 0, 2)).reshape(N_BKG, NR)
    return spikes_t, wr_full


def kernel(**inputs) -> np.ndarray:
    weights = np.asarray(inputs["weights"], dtype=np.float32)
    synaptic_weights = np.asarray(inputs["synaptic_weights"], dtype=np.float32)
    rest_of_brain = np.asarray(inputs["rest_of_brain"], dtype=np.float32)
    post_idx = np.asarray(inputs["post_idx"])
    pre_idx = np.asarray(inputs["pre_idx"])
    syn_ids = np.asarray(inputs["syn_ids"])

    spikes_t, wr_full = _host_preprocess(
        weights, synaptic_weights, rest_of_brain, post_idx, pre_idx, syn_ids
    )

    nc = get_nc()
    in_maps = [
        {
            "spikes_t": spikes_t,
            "wr": np.ascontiguousarray(
                wr_full[:, c * NR_CORE : (c + 1) * NR_CORE]
            ),
        }
        for c in range(N_CORES)
    ]
    res = run_bass_kernel_spmd(nc, in_maps, core_ids=list(range(N_CORES)))
    out = np.concatenate(
        [res.results[c]["out"] for c in range(N_CORES)], axis=1
    )                                                              # (600, 200000)
    return out.reshape(1, T, NR).astype(np.float32, copy=False)


# revision 5
# speedup vs baseline: 1.2836x; 1.2836x over previous
"""Trainium2 Bass kernel for nn_BackgroundNoiseLayer.

Math: out[t, n*5+r] = sum_k spikes[t,k] * Wr[k, n*5+r]
  spikes (600,100) binary, from rest_of_brain < 0.25
  Wr (100, 200000) = scatter-add of edge values (host-side index preprocessing)

Distribution: 1D column-parallel over the 8 cores — each core gets a
25000-wide slab of Wr (its 5000 post-neurons x 5 receptors), spikes
replicated; per-core output slabs (600, 25000) are concatenated on host.

Device kernel (SPMD, identical on all cores): fp32 precision is carried as
two bf16 matmuls (Wr = hi + lo, spikes exact in bf16) accumulating in fp32
PSUM — bf16 streams the PE at 1 cycle/row vs fp32's 4, so the pair is 2x
faster than one fp32 matmul and exact to ~2^-18. Per token tile
(128,128,128,128,88) stream matmul pairs (K=100, N=1000) into PSUM, copy
PSUM->SBUF staging alternating DVE/ACT, and DMA 2.5 MB staging blocks to the
DRAM output. HBM-write-bound (60 MB out per core): target_regime=memory.
"""

import numpy as np
import ml_dtypes

import concourse.bass as bass
import concourse.mybir as mybir
import concourse.tile as tile
from concourse.bass_utils import run_bass_kernel_spmd

BF16 = mybir.dt.bfloat16
F32 = mybir.dt.float32


# ---------------------------------------------------------------------------
# Workaround for walrus codegen limit on this toolchain: an instruction with
# more than one sync wait fails codegen ("Too many sync wait commands").
# Split every multi-wait instruction: extra waits move to single-wait NoOps
# inserted just before it on the same engine queue (same-engine FIFO dispatch
# preserves gating semantics).
# ---------------------------------------------------------------------------
def _split_multi_waits(nc):
    n_split = 0
    for fn in nc.m.functions:
        for bb in fn.blocks:
            new_list = []
            for inst in bb.instructions:
                si = inst.sync_info
                waits = list(si.on_wait) if si is not None and si.on_wait else []
                if len(waits) > 1:
                    for j, w in enumerate(waits[:-1]):
                        nop = mybir.InstNoOp(
                            name=f"{inst.name}_w{j}", ins=[], outs=[]
                        )
                        nop.engine = inst.engine
                        nop.sync_info = mybir.SyncInfo(on_wait=[w], on_update=[])
                        new_list.append(nop)
                        n_split += 1
                    inst.sync_info = mybir.SyncInfo(
                        on_wait=[waits[-1]], on_update=list(si.on_update or [])
                    )
                new_list.append(inst)
            bb.instructions = new_list
    return n_split


# ---------------------------------------------------------------------------
# Problem constants (hardcoded; kernel.py must be self-contained)
# ---------------------------------------------------------------------------
N_NEURONS = 40000
N_BKG = 100          # K (contraction dim)
N_SYN_BASIS = 5
T = 600              # BATCH * SEQ tokens
N_CORES = 8
NR = N_NEURONS * N_SYN_BASIS          # 200000 output columns
NR_CORE = NR // N_CORES               # 25000 per core

T_TILES = [128, 128, 128, 128, 88]    # sum = 600
CHUNK = 500                           # matmul N (psum bank limit: 512 fp32 out)
GROUP = 5000                          # staging width = 5 chunks
N_GROUPS = NR_CORE // GROUP           # 5

_NC_CACHE = None


def _build_nc():
    nc = bass.Bass()
    spikes_t = nc.dram_tensor("spikes_t", [N_BKG, T], BF16, kind="ExternalInput")
    wr_hi = nc.dram_tensor("wr_hi", [N_BKG, NR_CORE], BF16, kind="ExternalInput")
    wr_lo = nc.dram_tensor("wr_lo", [N_BKG, NR_CORE], BF16, kind="ExternalInput")
    out = nc.dram_tensor("out", [T, NR_CORE], F32, kind="ExternalOutput")

    with tile.TileContext(nc) as tc:
        with (
            tc.tile_pool(name="wpool", bufs=1) as wpool,
            tc.tile_pool(name="spool", bufs=1) as spool,
            tc.tile_pool(name="stage", bufs=3) as stage,
            tc.tile_pool(name="psum", bufs=8, space="PSUM") as psum,
        ):
            sp_sb = spool.tile([N_BKG, T], BF16)
            nc.sync.dma_start(sp_sb[:], spikes_t[:])
            w_sb = []                      # per-group (hi, lo) SBUF tiles
            for g in range(N_GROUPS):
                gh = wpool.tile([N_BKG, GROUP], BF16, tag=f"wh{g}")
                gl = wpool.tile([N_BKG, GROUP], BF16, tag=f"wl{g}")
                sl = slice(g * GROUP, (g + 1) * GROUP)
                nc.sync.dma_start(gh[:], wr_hi[:, sl])
                nc.sync.dma_start(gl[:], wr_lo[:, sl])
                w_sb.append((gh, gl))

            copy_i = 0
            for ti, m in enumerate(T_TILES):
                t0 = ti * 128
                lhs = sp_sb[:, t0 : t0 + m]
                for g in range(N_GROUPS):
                    gh, gl = w_sb[g]
                    st = stage.tile([m, GROUP], F32)
                    for c in range(GROUP // CHUNK):
                        cs = slice(c * CHUNK, (c + 1) * CHUNK)
                        ps = psum.tile([m, CHUNK], F32)
                        nc.tensor.matmul(
                            ps[:], lhs, gh[:, cs], start=True, stop=False
                        )
                        nc.tensor.matmul(
                            ps[:], lhs, gl[:, cs], start=False, stop=True
                        )
                        if copy_i % 2 == 0:
                            nc.vector.tensor_copy(st[:, cs], ps[:])
                        else:
                            nc.scalar.copy(st[:, cs], ps[:])
                        copy_i += 1
                    nc.scalar.dma_start(
                        out[t0 : t0 + m, g * GROUP : (g + 1) * GROUP], st[:]
                    )
    _split_multi_waits(nc)
    return nc


def get_nc():
    global _NC_CACHE
    if _NC_CACHE is None:
        _NC_CACHE = _build_nc()
    return _NC_CACHE


def _host_preprocess(weights, synaptic_weights, rest_of_brain, post_idx, pre_idx,
                     syn_ids):
    spikes = (rest_of_brain.reshape(T, N_BKG) < 0.25).astype(np.float32)
    spikes_t = np.ascontiguousarray(spikes.T).astype(ml_dtypes.bfloat16)

    vals = weights[:, None] * synaptic_weights[syn_ids]            # (nnz, 5)
    cell = post_idx.astype(np.int64) * N_BKG + pre_idx.astype(np.int64)
    flat = (cell[:, None] * N_SYN_BASIS + np.arange(N_SYN_BASIS)[None, :]).ravel()
    w_dense = np.bincount(
        flat, weights=vals.astype(np.float64).ravel(),
        minlength=N_NEURONS * N_BKG * N_SYN_BASIS,
    ).astype(np.float32).reshape(N_NEURONS, N_BKG, N_SYN_BASIS)
    # Wr[k, n*5+r] = W[n, k, r]; split fp32 into bf16 hi + lo for the PE
    wr_full = np.ascontiguousarray(w_dense.transpose(1, 0, 2)).reshape(N_BKG, NR)
    wr_hi = wr_full.astype(ml_dtypes.bfloat16)
    wr_lo = (wr_full - wr_hi.astype(np.float32)).astype(ml_dtypes.bfloat16)
    return spikes_t, wr_hi, wr_lo


def kernel(**inputs) -> np.ndarray:
    weights = np.asarray(inputs["weights"], dtype=np.float32)
    synaptic_weights = np.asarray(inputs["synaptic_weights"], dtype=np.float32)
    rest_of_brain = np.asarray(inputs["rest_of_brain"], dtype=np.float32)
    post_idx = np.asarray(inputs["post_idx"])
    pre_idx = np.asarray(inputs["pre_idx"])
    syn_ids = np.asarray(inputs["syn_ids"])

    spikes_t, wr_hi, wr_lo = _host_preprocess(
        weights, synaptic_weights, rest_of_brain, post_idx, pre_idx, syn_ids
    )

    nc = get_nc()
    in_maps = [
        {
            "spikes_t": spikes_t,
            "wr_hi": np.ascontiguousarray(wr_hi[:, c * NR_CORE : (c + 1) * NR_CORE]),
            "wr_lo": np.ascontiguousarray(wr_lo[:, c * NR_CORE : (c + 1) * NR_CORE]),
        }
        for c in range(N_CORES)
    ]
    res = run_bass_kernel_spmd(nc, in_maps, core_ids=list(range(N_CORES)))
    out = np.concatenate(
        [res.results[c]["out"] for c in range(N_CORES)], axis=1
    )                                                              # (600, 200000)
    return out.reshape(1, T, NR).astype(np.float32, copy=False)
